# revision 22
# baseline (speedup 1.0000x reference)
"""Trainium2 Bass kernel for nn_AttentionBlock (B=8, C=512, T=2048, K=V=512).

Data-parallel over batch: 8 batch elements -> 8 NeuronCores, no collectives.

Per core (one batch element, x_b is (C, T) in DRAM):
  qT (K, T) = Wq @ x_b + bq      (stored k-major: partition = k % 128)
  kT (K, T) = Wk @ x_b + bk
  v  (T, V) = x_b^T @ Wv^T + bv  (partition = t % 128)
  S^T (j, i) = k @ q^T           computed only for valid columns i >= j (causal)
  W^T = softmax over i (free axis) of S^T / sqrt(K)  [reference softmaxes the
        QUERY axis, which in the transposed layout is the free axis]
  out^T (V, T) = sum_j v[j]^T * W^T[j]   (PSUM accumulation, causal skip)
  weights (i, j) = PE-transpose of W^T tiles, 4 per PSUM bank, batched copies;
        the masked region j > i is never written (output buffers are pre-zeroed
        by run_bass_kernel_spmd / the PJRT donation path).

Weight matrices are transposed on the host (numpy) so no on-device transpose
is needed for the projections. All matmuls run in float32r (TF32-like, full PE
rate at moving free dim >= 256).

A post-pass splits excess semaphore waits onto EventSemaphore instructions:
the walrus build here allows only one sync wait per instruction and the Tile
scheduler does not know that.
"""
import json
import math

import numpy as np

B, C, T = 8, 512, 2048
KS = 512          # KEY_SIZE == VALUE_SIZE
NCORES = 8
P = 128           # partitions
CN = C // P       # 4 contraction chunks
KM = KS // P      # 4 k/v row tiles
TN = T // P       # 16 t tiles (j blocks)
NCH = T // 512    # 4 column chunks of 512
SCALE = 1.0 / math.sqrt(KS)
NEG = -1.0e30

_CACHED = {}


def _legalize_waits(bir_json_bytes: bytes) -> bytes:
    """Split multi-wait instructions: walrus allows 1 sync wait per instruction."""
    d = json.loads(bir_json_bytes)
    n_new = 0
    for f in d["functions"]:
        for bb in f["blocks"]:
            out = []
            for ins in bb["instructions"]:
                si = ins.get("sync_info") or {}
                waits = si.get("on_wait") or []
                if len(waits) > 1:
                    excess = waits[:-1]
                    si["on_wait"] = waits[-1:]
                    for w in excess:
                        n_new += 1
                        out.append({
                            "debug": ins.get("debug"),
                            "engine": ins["engine"],
                            "ins": [],
                            "name": f"evw-{n_new}",
                            "opcode": "EventSemaphore",
                            "outs": [],
                            "sync_info": {"on_update": [], "on_wait": [w]},
                        })
                out.append(ins)
            bb["instructions"] = out
    return json.dumps(d).encode()


def _build_nc():
    import concourse.bass as bass
    import concourse.mybir as mybir
    from concourse.masks import make_identity
    from concourse.tile import TileContext

    dt = mybir.dt
    f32 = dt.float32
    f32r = dt.float32r
    AF = mybir.ActivationFunctionType
    OP = mybir.AluOpType

    nc = bass.Bass()
    # inputs: x natural layout; weight matrices pre-transposed on host to (C, K)
    x_d = nc.dram_tensor("x", [C, T], f32r, kind="ExternalInput")
    wqT_d = nc.dram_tensor("WqT", [C, KS], f32r, kind="ExternalInput")
    wkT_d = nc.dram_tensor("WkT", [C, KS], f32r, kind="ExternalInput")
    wvT_d = nc.dram_tensor("WvT", [C, KS], f32r, kind="ExternalInput")
    bq_d = nc.dram_tensor("bq", [KS], f32, kind="ExternalInput")
    bk_d = nc.dram_tensor("bk", [KS], f32, kind="ExternalInput")
    bv_d = nc.dram_tensor("bv", [KS], f32, kind="ExternalInput")
    out_d = nc.dram_tensor("out", [KS, T], f32, kind="ExternalOutput")
    wts_d = nc.dram_tensor("weights", [T, T], f32, kind="ExternalOutput")

    with TileContext(nc) as tc:
        with tc.tile_pool(name="consts", bufs=1) as consts, \
             tc.tile_pool(name="vpool", bufs=1) as vpool, \
             tc.tile_pool(name="qkpool", bufs=1) as qkpool, \
             tc.tile_pool(name="ps_t", bufs=2, space="PSUM") as ps_t:
            # ---- constants -------------------------------------------------
            ident_f = consts.tile([P, P], f32)
            make_identity(nc, ident_f)
            ident = consts.tile([P, P], f32r)
            nc.vector.tensor_copy(out=ident, in_=ident_f)
            # causal mask for the diagonal band of S^T (rows j, cols i):
            # keep where i >= j, else -1e30
            cmask = consts.tile([P, P], f32)
            nc.gpsimd.memset(cmask, 0.0)
            nc.gpsimd.affine_select(
                out=cmask, in_=cmask,
                compare_op=OP.is_ge, fill=NEG,
                base=0, pattern=[[1, P]], channel_multiplier=-1,
            )
            bq_sb = consts.tile([P, KM], f32)
            bk_sb = consts.tile([P, KM], f32)
            nc.sync.dma_start(out=bq_sb, in_=bq_d.ap().rearrange("(m p) -> p m", p=P))
            nc.sync.dma_start(out=bk_sb, in_=bk_d.ap().rearrange("(m p) -> p m", p=P))
            bv_bc = consts.tile([P, KS], f32)
            nc.gpsimd.dma_start(
                out=bv_bc,
                in_=bass.AP(tensor=bv_d, offset=0, ap=[[0, P], [1, KS]]),
            )

            qT = qkpool.tile([P, KM, T], f32r, tag="qT")
            kT = qkpool.tile([P, KM, T], f32r, tag="kT")
            v_sb = vpool.tile([P, TN, KS], f32r, tag="v")

            with tc.tile_pool(name="wpool", bufs=1) as wpool_early, \
                 tc.tile_pool(name="small", bufs=6) as small, \
                 tc.tile_pool(name="stage", bufs=2) as stage, \
                 tc.tile_pool(name="ps_s", bufs=4, space="PSUM") as ps_s, \
                 tc.tile_pool(name="ps_o", bufs=2, space="PSUM") as ps_o:
                w_t = []   # per-jb softmaxed W^T tiles
                state = {"grp": 0, "wpool": wpool_early}

                def scores_block(jb):
                    """scores + softmax for j-block jb."""
                    j0 = jb * P
                    c0 = j0 // 512
                    width = T - j0
                    wtile = state["wpool"].tile([P, width], f32, tag=f"w{jb}")
                    w_t.append(wtile)
                    nchunks = NCH - c0
                    acc = small.tile([P, nchunks], f32, tag="acc")
                    for ic in range(c0, NCH):
                        a = max(ic * 512, j0)          # global col start
                        bnd = (ic + 1) * 512           # global col end
                        w = bnd - a
                        ps = ps_s.tile([P, 512], f32, tag="sc")
                        for d in range(CN):
                            nc.tensor.matmul(
                                ps[:, :w],
                                kT[:, d, j0:j0 + P],
                                qT[:, d, a:bnd],
                                start=(d == 0), stop=(d == CN - 1),
                            )
                        if ic == c0:
                            nc.vector.tensor_add(
                                out=ps[:, :P], in0=ps[:, :P], in1=cmask)
                        nc.scalar.activation(
                            out=wtile.bitcast(f32r)[:, a - j0:bnd - j0],
                            in_=ps[:, :w],
                            func=AF.Exp, scale=SCALE,
                            accum_out=acc[:, ic - c0:ic - c0 + 1],
                        )
                    ssum = small.tile([P, 1], f32, tag="ssum")
                    nc.vector.reduce_sum(
                        out=ssum, in_=acc, axis=mybir.AxisListType.X)
                    rcp = small.tile([P, 1], f32, tag="rcp")
                    nc.vector.reciprocal(out=rcp, in_=ssum)
                    nc.vector.tensor_scalar_mul(
                        out=wtile.bitcast(f32r), in0=wtile.bitcast(f32r),
                        scalar1=rcp)

                def wout_col(jj):
                    """weights-output column block jj: rows i in [jj*128, T),
                    cols j in [jj*128, (jj+1)*128). Needs only w_t[jj]."""
                    jj0 = jj * P
                    nt = TN - jj
                    stg = stage.tile([P, nt, P], f32, tag="stg")
                    for g0 in range(0, nt, 4):
                        gn = min(4, nt - g0)
                        pt = ps_t.tile([P, gn * P], f32r, tag="tr")
                        for gi in range(gn):
                            it = g0 + gi          # i-tile index rel to jj
                            i0 = jj0 + it * P
                            nc.tensor.transpose(
                                pt[:, gi * P:(gi + 1) * P],
                                w_t[jj].bitcast(f32r)[:, i0 - jj0:i0 - jj0 + P],
                                ident)
                        if state["grp"] % 2 == 0:
                            nc.vector.tensor_copy(
                                out=stg[:, g0:g0 + gn, :], in_=pt)
                        else:
                            nc.scalar.copy(
                                out=stg[:, g0:g0 + gn, :], in_=pt)
                        state["grp"] += 1
                    nc.sync.dma_start(
                        out=wts_d.ap()[jj0:T, jj0:jj0 + P].rearrange(
                            "(it p) j -> p it j", p=P),
                        in_=stg)

                def out_burst(ic):
                    """out^T chunk ic = sum over j blocks 0..4(ic+1)-1."""
                    jmax = 4 * (ic + 1)
                    for m in range(KM):
                        po = ps_o.tile([P, 512], f32, tag="o")
                        for jj in range(jmax):
                            jj0 = jj * P
                            a = max(ic * 512, jj0)
                            bnd = (ic + 1) * 512
                            nc.tensor.matmul(
                                po[:, a - ic * 512:512],
                                v_sb[:, jj, m * P:(m + 1) * P],
                                w_t[jj].bitcast(f32r)[:, a - jj0:bnd - jj0],
                                start=(jj == 0), stop=(jj == jmax - 1),
                            )
                        o_sb = stage.tile([P, 512], f32, tag="osb")
                        nc.scalar.copy(out=o_sb, in_=po)
                        nc.sync.dma_start(
                            out=out_d.ap()[m * P:(m + 1) * P,
                                           ic * 512:(ic + 1) * 512],
                            in_=o_sb)

                with tc.tile_pool(name="loads", bufs=1) as loads:
                    # ---- load x (per column chunk) and weights -------------
                    x_sb = loads.tile([P, CN, T], f32r)
                    xr = x_d.rearrange("(c p) t -> p c t", p=P)
                    wT = {}
                    for name, w_d in (("q", wqT_d), ("k", wkT_d)):
                        wt = loads.tile([P, CN, KS], f32r, tag=f"wT{name}")
                        wT[name] = wt
                        nc.sync.dma_start(
                            out=wt, in_=w_d.rearrange("(c p) k -> p c k", p=P))
                    for n in range(NCH):
                        nc.sync.dma_start(
                            out=x_sb[:, :, n * 512:(n + 1) * 512],
                            in_=xr[:, :, n * 512:(n + 1) * 512])
                    wtv = loads.tile([P, CN, KS], f32r, tag="wTv")
                    nc.sync.dma_start(
                        out=wtv, in_=wvT_d.rearrange("(c p) k -> p c k", p=P))

                    # ---- q/k projections -----------------------------------
                    for dst, wt, b_sb in ((qT, wT["q"], bq_sb),
                                          (kT, wT["k"], bk_sb)):
                        for m in range(KM):
                            for n in range(NCH):
                                ps = ps_s.tile([P, 512], f32, tag="sc")
                                for c in range(CN):
                                    nc.tensor.matmul(
                                        ps,
                                        wt[:, c, m * P:(m + 1) * P],
                                        x_sb[:, c, n * 512:(n + 1) * 512],
                                        start=(c == 0), stop=(c == CN - 1),
                                    )
                                nc.scalar.add(
                                    out=dst[:, m, n * 512:(n + 1) * 512],
                                    in_=ps, add=b_sb[:, m:m + 1],
                                )

                    # scores for the first 4 j-blocks don't need v
                    for jb in range(4):
                        scores_block(jb)
                        wout_col(jb)

                    # ---- v projection --------------------------------------
                    for t in range(TN):
                        ps = ps_s.tile([P, 512], f32, tag="sc")
                        for c in range(CN):
                            nc.tensor.matmul(
                                ps,
                                x_sb[:, c, t * P:(t + 1) * P],
                                wtv[:, c, :],
                                start=(c == 0), stop=(c == CN - 1),
                            )
                        nc.vector.tensor_add(out=v_sb[:, t, :], in0=ps,
                                             in1=bv_bc)
                    out_burst(0)

                with tc.tile_pool(name="wpool2", bufs=1) as wpool_late:
                    state["wpool"] = wpool_late
                    for jb in range(4, TN):
                        scores_block(jb)
                        wout_col(jb)
                        if jb % 4 == 3:
                            out_burst(jb // 4)

    return nc


def _get_nc():
    if "nc" not in _CACHED:
        nc = _build_nc()
        orig = nc.to_json_bytes
        nc.to_json_bytes = lambda: _legalize_waits(orig())
        _CACHED["nc"] = nc
    return _CACHED["nc"]


PROFILE = {"trace": False, "result": None}


def kernel(**inputs):
    from concourse.bass_utils import run_bass_kernel_spmd

    x = np.ascontiguousarray(np.asarray(inputs["x"], dtype=np.float32))
    shared = {
        "WqT": np.ascontiguousarray(np.asarray(inputs["Wq"], np.float32).T),
        "WkT": np.ascontiguousarray(np.asarray(inputs["Wk"], np.float32).T),
        "WvT": np.ascontiguousarray(np.asarray(inputs["Wv"], np.float32).T),
        "bq": np.ascontiguousarray(np.asarray(inputs["bq"], np.float32)),
        "bk": np.ascontiguousarray(np.asarray(inputs["bk"], np.float32)),
        "bv": np.ascontiguousarray(np.asarray(inputs["bv"], np.float32)),
    }
    nc = _get_nc()
    in_maps = [dict(shared, x=x[b]) for b in range(B)]
    res = run_bass_kernel_spmd(
        nc, in_maps, core_ids=list(range(NCORES)),
        trace=PROFILE["trace"],
    )
    PROFILE["result"] = res
    out = np.stack([r["out"] for r in res.results])
    wts = np.stack([r["weights"] for r in res.results])
    return out, wts


# revision 23
# speedup vs baseline: 1.0167x; 1.0167x over previous
"""Trainium2 Bass kernel for nn_AttentionBlock (B=8, C=512, T=2048, K=V=512).

Data-parallel over batch: 8 batch elements -> 8 NeuronCores, no collectives.

Per core (one batch element, x_b is (C, T) in DRAM):
  qT (K, T) = Wq @ x_b + bq      (stored k-major: partition = k % 128)
  kT (K, T) = Wk @ x_b + bk
  v  (T, V) = x_b^T @ Wv^T + bv  (partition = t % 128)
  S^T (j, i) = k @ q^T           computed only for valid columns i >= j (causal)
  W^T = softmax over i (free axis) of S^T / sqrt(K)  [reference softmaxes the
        QUERY axis, which in the transposed layout is the free axis]
  out^T (V, T) = sum_j v[j]^T * W^T[j]   (PSUM accumulation, causal skip)
  weights (i, j) = PE-transpose of W^T tiles, 4 per PSUM bank, batched copies;
        the masked region j > i is never written (output buffers are pre-zeroed
        by run_bass_kernel_spmd / the PJRT donation path).

Weight matrices are transposed on the host (numpy) so no on-device transpose
is needed for the projections. All matmuls run in float32r (TF32-like, full PE
rate at moving free dim >= 256).

A post-pass splits excess semaphore waits onto EventSemaphore instructions:
the walrus build here allows only one sync wait per instruction and the Tile
scheduler does not know that.
"""
import json
import math

import numpy as np

B, C, T = 8, 512, 2048
KS = 512          # KEY_SIZE == VALUE_SIZE
NCORES = 8
P = 128           # partitions
CN = C // P       # 4 contraction chunks
KM = KS // P      # 4 k/v row tiles
TN = T // P       # 16 t tiles (j blocks)
NCH = T // 512    # 4 column chunks of 512
SCALE = 1.0 / math.sqrt(KS)
NEG = -1.0e30

_CACHED = {}


def _legalize_waits(bir_json_bytes: bytes) -> bytes:
    """Split multi-wait instructions: walrus allows 1 sync wait per instruction."""
    d = json.loads(bir_json_bytes)
    n_new = 0
    for f in d["functions"]:
        for bb in f["blocks"]:
            out = []
            for ins in bb["instructions"]:
                si = ins.get("sync_info") or {}
                waits = si.get("on_wait") or []
                if len(waits) > 1:
                    excess = waits[:-1]
                    si["on_wait"] = waits[-1:]
                    for w in excess:
                        n_new += 1
                        out.append({
                            "debug": ins.get("debug"),
                            "engine": ins["engine"],
                            "ins": [],
                            "name": f"evw-{n_new}",
                            "opcode": "EventSemaphore",
                            "outs": [],
                            "sync_info": {"on_update": [], "on_wait": [w]},
                        })
                out.append(ins)
            bb["instructions"] = out
    return json.dumps(d).encode()


def _build_nc():
    import concourse.bass as bass
    import concourse.mybir as mybir
    from concourse.masks import make_identity
    from concourse.tile import TileContext

    dt = mybir.dt
    f32 = dt.float32
    f32r = dt.float32r
    AF = mybir.ActivationFunctionType
    OP = mybir.AluOpType

    nc = bass.Bass()
    # inputs: x natural layout; weight matrices pre-transposed on host to (C, K)
    x_d = nc.dram_tensor("x", [C, T], f32r, kind="ExternalInput")
    wqT_d = nc.dram_tensor("WqT", [C, KS], f32r, kind="ExternalInput")
    wkT_d = nc.dram_tensor("WkT", [C, KS], f32r, kind="ExternalInput")
    wvT_d = nc.dram_tensor("WvT", [C, KS], f32r, kind="ExternalInput")
    bq_d = nc.dram_tensor("bq", [KS], f32, kind="ExternalInput")
    bk_d = nc.dram_tensor("bk", [KS], f32, kind="ExternalInput")
    bv_d = nc.dram_tensor("bv", [KS], f32, kind="ExternalInput")
    out_d = nc.dram_tensor("out", [KS, T], f32, kind="ExternalOutput")
    wts_d = nc.dram_tensor("weights", [T, T], f32, kind="ExternalOutput")

    with TileContext(nc) as tc:
        with tc.tile_pool(name="consts", bufs=1) as consts, \
             tc.tile_pool(name="vpool", bufs=1) as vpool, \
             tc.tile_pool(name="qkpool", bufs=1) as qkpool, \
             tc.tile_pool(name="ps_t", bufs=2, space="PSUM") as ps_t:
            # ---- constants -------------------------------------------------
            ident_f = consts.tile([P, P], f32)
            make_identity(nc, ident_f)
            ident = consts.tile([P, P], f32r)
            nc.vector.tensor_copy(out=ident, in_=ident_f)
            # causal mask for the diagonal band of S^T (rows j, cols i):
            # keep where i >= j, else -1e30
            cmask = consts.tile([P, P], f32)
            nc.gpsimd.memset(cmask, 0.0)
            nc.gpsimd.affine_select(
                out=cmask, in_=cmask,
                compare_op=OP.is_ge, fill=NEG,
                base=0, pattern=[[1, P]], channel_multiplier=-1,
            )
            bq_sb = consts.tile([P, KM], f32)
            bk_sb = consts.tile([P, KM], f32)
            nc.sync.dma_start(out=bq_sb, in_=bq_d.ap().rearrange("(m p) -> p m", p=P))
            nc.sync.dma_start(out=bk_sb, in_=bk_d.ap().rearrange("(m p) -> p m", p=P))
            bv_bc = consts.tile([P, KS], f32)
            nc.gpsimd.dma_start(
                out=bv_bc,
                in_=bass.AP(tensor=bv_d, offset=0, ap=[[0, P], [1, KS]]),
            )

            qT = qkpool.tile([P, KM, T], f32r, tag="qT")
            kT = qkpool.tile([P, KM, T], f32r, tag="kT")
            v_sb = vpool.tile([P, TN, KS], f32r, tag="v")

            with tc.tile_pool(name="wpool", bufs=1) as wpool_early, \
                 tc.tile_pool(name="small", bufs=6) as small, \
                 tc.tile_pool(name="stage", bufs=2) as stage, \
                 tc.tile_pool(name="ps_s", bufs=4, space="PSUM") as ps_s, \
                 tc.tile_pool(name="ps_o", bufs=2, space="PSUM") as ps_o:
                w_t = []   # per-jb softmaxed W^T tiles
                state = {"grp": 0, "wpool": wpool_early}

                def scores_block(jb):
                    """scores + softmax for j-block jb."""
                    j0 = jb * P
                    c0 = j0 // 512
                    width = T - j0
                    wtile = state["wpool"].tile([P, width], f32, tag=f"w{jb}")
                    w_t.append(wtile)
                    nchunks = NCH - c0
                    acc = small.tile([P, nchunks], f32, tag="acc")
                    for ic in range(c0, NCH):
                        a = max(ic * 512, j0)          # global col start
                        bnd = (ic + 1) * 512           # global col end
                        w = bnd - a
                        ps = ps_s.tile([P, 512], f32, tag="sc")
                        for d in range(CN):
                            nc.tensor.matmul(
                                ps[:, :w],
                                kT[:, d, j0:j0 + P],
                                qT[:, d, a:bnd],
                                start=(d == 0), stop=(d == CN - 1),
                            )
                        if ic == c0:
                            nc.vector.tensor_add(
                                out=ps[:, :P], in0=ps[:, :P], in1=cmask)
                        nc.scalar.activation(
                            out=wtile.bitcast(f32r)[:, a - j0:bnd - j0],
                            in_=ps[:, :w],
                            func=AF.Exp, scale=SCALE,
                            accum_out=acc[:, ic - c0:ic - c0 + 1],
                        )
                    ssum = small.tile([P, 1], f32, tag="ssum")
                    nc.vector.reduce_sum(
                        out=ssum, in_=acc, axis=mybir.AxisListType.X)
                    rcp = small.tile([P, 1], f32, tag="rcp")
                    nc.vector.reciprocal(out=rcp, in_=ssum)
                    nc.vector.tensor_scalar_mul(
                        out=wtile.bitcast(f32r), in0=wtile.bitcast(f32r),
                        scalar1=rcp)

                def wout_col(jj):
                    """weights-output column block jj: rows i in [jj*128, T),
                    cols j in [jj*128, (jj+1)*128). Needs only w_t[jj]."""
                    jj0 = jj * P
                    nt = TN - jj
                    stg = stage.tile([P, nt, P], f32, tag="stg")
                    for g0 in range(0, nt, 4):
                        gn = min(4, nt - g0)
                        pt = ps_t.tile([P, gn * P], f32r, tag="tr")
                        for gi in range(gn):
                            it = g0 + gi          # i-tile index rel to jj
                            i0 = jj0 + it * P
                            nc.tensor.transpose(
                                pt[:, gi * P:(gi + 1) * P],
                                w_t[jj].bitcast(f32r)[:, i0 - jj0:i0 - jj0 + P],
                                ident)
                        if state["grp"] % 2 == 0:
                            nc.vector.tensor_copy(
                                out=stg[:, g0:g0 + gn, :], in_=pt)
                        else:
                            nc.scalar.copy(
                                out=stg[:, g0:g0 + gn, :], in_=pt)
                        state["grp"] += 1
                    nc.sync.dma_start(
                        out=wts_d.ap()[jj0:T, jj0:jj0 + P].rearrange(
                            "(it p) j -> p it j", p=P),
                        in_=stg)

                def out_burst(ic):
                    """out^T chunk ic = sum over j blocks 0..4(ic+1)-1."""
                    jmax = 4 * (ic + 1)
                    for m in range(KM):
                        po = ps_o.tile([P, 512], f32, tag="o")
                        for jj in range(jmax):
                            jj0 = jj * P
                            a = max(ic * 512, jj0)
                            bnd = (ic + 1) * 512
                            nc.tensor.matmul(
                                po[:, a - ic * 512:512],
                                v_sb[:, jj, m * P:(m + 1) * P],
                                w_t[jj].bitcast(f32r)[:, a - jj0:bnd - jj0],
                                start=(jj == 0), stop=(jj == jmax - 1),
                            )
                        o_sb = stage.tile([P, 512], f32, tag="osb")
                        nc.scalar.copy(out=o_sb, in_=po)
                        nc.sync.dma_start(
                            out=out_d.ap()[m * P:(m + 1) * P,
                                           ic * 512:(ic + 1) * 512],
                            in_=o_sb)

                with tc.tile_pool(name="loads", bufs=1) as loads:
                    # ---- load x (per column chunk) and weights -------------
                    x_sb = loads.tile([P, CN, T], f32r)
                    xr = x_d.rearrange("(c p) t -> p c t", p=P)
                    wT = {}
                    wq_t = loads.tile([P, CN, KS], f32r, tag="wTq")
                    wT["q"] = wq_t
                    nc.sync.dma_start(
                        out=wq_t, in_=wqT_d.rearrange("(c p) k -> p c k", p=P))
                    for n in range(NCH):
                        nc.sync.dma_start(
                            out=x_sb[:, :, n * 512:(n + 1) * 512],
                            in_=xr[:, :, n * 512:(n + 1) * 512])
                    wk_t = loads.tile([P, CN, KS], f32r, tag="wTk")
                    wT["k"] = wk_t
                    nc.sync.dma_start(
                        out=wk_t, in_=wkT_d.rearrange("(c p) k -> p c k", p=P))
                    wtv = loads.tile([P, CN, KS], f32r, tag="wTv")
                    nc.sync.dma_start(
                        out=wtv, in_=wvT_d.rearrange("(c p) k -> p c k", p=P))

                    # ---- q/k projections -----------------------------------
                    for dst, wt, b_sb in ((qT, wT["q"], bq_sb),
                                          (kT, wT["k"], bk_sb)):
                        for m in range(KM):
                            for n in range(NCH):
                                ps = ps_s.tile([P, 512], f32, tag="sc")
                                for c in range(CN):
                                    nc.tensor.matmul(
                                        ps,
                                        wt[:, c, m * P:(m + 1) * P],
                                        x_sb[:, c, n * 512:(n + 1) * 512],
                                        start=(c == 0), stop=(c == CN - 1),
                                    )
                                nc.scalar.add(
                                    out=dst[:, m, n * 512:(n + 1) * 512],
                                    in_=ps, add=b_sb[:, m:m + 1],
                                )

                    # scores for the first 4 j-blocks don't need v
                    for jb in range(4):
                        scores_block(jb)
                        wout_col(jb)

                    # ---- v projection --------------------------------------
                    for t in range(TN):
                        ps = ps_s.tile([P, 512], f32, tag="sc")
                        for c in range(CN):
                            nc.tensor.matmul(
                                ps,
                                x_sb[:, c, t * P:(t + 1) * P],
                                wtv[:, c, :],
                                start=(c == 0), stop=(c == CN - 1),
                            )
                        nc.vector.tensor_add(out=v_sb[:, t, :], in0=ps,
                                             in1=bv_bc)
                    out_burst(0)

                with tc.tile_pool(name="wpool2", bufs=1) as wpool_late:
                    state["wpool"] = wpool_late
                    for jb in range(4, TN):
                        scores_block(jb)
                        wout_col(jb)
                        if jb % 4 == 3:
                            out_burst(jb // 4)

    return nc


def _get_nc():
    if "nc" not in _CACHED:
        nc = _build_nc()
        orig = nc.to_json_bytes
        nc.to_json_bytes = lambda: _legalize_waits(orig())
        _CACHED["nc"] = nc
    return _CACHED["nc"]


PROFILE = {"trace": False, "result": None}


def kernel(**inputs):
    from concourse.bass_utils import run_bass_kernel_spmd

    x = np.ascontiguousarray(np.asarray(inputs["x"], dtype=np.float32))
    shared = {
        "WqT": np.ascontiguousarray(np.asarray(inputs["Wq"], np.float32).T),
        "WkT": np.ascontiguousarray(np.asarray(inputs["Wk"], np.float32).T),
        "WvT": np.ascontiguousarray(np.asarray(inputs["Wv"], np.float32).T),
        "bq": np.ascontiguousarray(np.asarray(inputs["bq"], np.float32)),
        "bk": np.ascontiguousarray(np.asarray(inputs["bk"], np.float32)),
        "bv": np.ascontiguousarray(np.asarray(inputs["bv"], np.float32)),
    }
    nc = _get_nc()
    in_maps = [dict(shared, x=x[b]) for b in range(B)]
    res = run_bass_kernel_spmd(
        nc, in_maps, core_ids=list(range(NCORES)),
        trace=PROFILE["trace"],
    )
    PROFILE["result"] = res
    out = np.stack([r["out"] for r in res.results])
    wts = np.stack([r["weights"] for r in res.results])
    return out, wts
